# revision 9
# baseline (speedup 1.0000x reference)
"""PolyMatchingLoss Trainium2 kernel.

Reference computation (B=128, P=1024, C=2):
    dis[b, i] = mean_j sum_c smooth_l1(pred[b,j,c] - gt[b,(i+j)%P,c])
    out = mean_b min_i dis[b, i]

Strategy:
  - Pure data parallel over batch: 16 batches per core x 8 cores.
  - Per (b, shift-block qi): one fused custom DVE instruction computes
    2*smooth_l1(W - P) elementwise over a [128 shifts, 2048 (j,c)] tile
    and reduces (sum) along the free axis into a [128,1] accumulator
    column.  smooth_l1 via m*(2t-m) = 2*huber, t=|d|, m=min(t,1).
  - The gt operand uses the "staircase" identity: W[x, y] = gtflat2[2x+y]
    (c-interleaved, cyclically duplicated gt).  One [128, 3840] window
    tile per (b) serves all 8 shift blocks as free-axis offset slices.
  - pred is broadcast along partitions (host-replicated).
  - min over shifts + mean over batch on host (tiny).
"""

from operator import add as _operator_add

import numpy as np
import ml_dtypes

_bf16 = ml_dtypes.bfloat16

from concourse import mybir
from concourse import bass, bass_utils
from concourse.tile import TileContext
import concourse.dve_ops as _dve_ops
from concourse.dve_ops import DveOp
from concourse.dve_spec import Spec, Src0, Src1, Zero, One, maxx, minn

# ---------------------------------------------------------------------------
# Workaround: this toolchain's walrus allows at most ONE sync wait per
# instruction; Tile emits 2+.  Split extras onto EventSemaphore carrier
# instructions inserted just before the offending instruction.
# ---------------------------------------------------------------------------
def _split_multi_waits(nc) -> int:
    n = 0
    for fn in nc.m.functions:
        for bb in fn.blocks:
            out = []
            for inst in bb.instructions:
                si = inst.sync_info
                if si is not None and si.on_wait and len(si.on_wait) > 1:
                    for k, w in enumerate(si.on_wait[:-1]):
                        out.append(
                            mybir.InstEventSemaphore(
                                name=f"{inst.name}_wsplit{k}",
                                opcode="EventSemaphore",
                                engine=inst.engine,
                                ins=[],
                                outs=[],
                                sync_info=mybir.SyncInfo(on_wait=[w], on_update=[]),
                            )
                        )
                        n += 1
                    si.on_wait = [si.on_wait[-1]]
                out.append(inst)
            bb.instructions = out
    return n


B = 128
PNUM = 1024
C = 2
NCORES = 8
BL = B // NCORES  # batches per core
FD = PNUM * C  # 2048 free elements per tile
WW = FD + 256 * 7  # 3840 window width


# --------------------------------------------------------------------------
# Custom DVE op: out = m*(2t - m) with t=|in0-in1|, m=min(t,1)  (= 2*huber)
#                accum_out = sum over free axis
# --------------------------------------------------------------------------
def _huber_ref(in0, in1, s0, s1, imm2):
    dd = in0.astype(np.float32) - in1.astype(np.float32)
    tt = np.abs(dd)
    mm = np.minimum(tt, 1.0)
    bb = (mm * (2.0 * tt - mm)).astype(np.float32)
    return bb, bb.reshape(bb.shape[0], -1).sum(axis=-1, keepdims=True)


def _make_huber_op() -> DveOp:
    d = Src0 - Src1
    nd = Src1 - Src0
    t = maxx(d, nd)
    m = minn(t, One)
    v = t - m
    w = t + v
    body = m * w
    return DveOp(
        "TENSOR_HUBER2_REDUCE",
        Spec(body=body, accum=_operator_add, accum_init=Zero, reference=_huber_ref),
        subdim=False,
        uops_sha={"v3": "e8f6160a1f1db788", "v4": "8b26f7daea78cb80"},
    )


def _register_op(op: DveOp) -> None:
    if op.name in _dve_ops._SUB_OPCODE_FOR_NAME:
        return
    _dve_ops.OPS.append(op)
    _dve_ops._SUB_OPCODE_FOR_NAME[op.name] = (
        _dve_ops._CUSTOM_DVE_ROW_BASE + len(_dve_ops.OPS) - 1
    )
    _dve_ops.CUSTOM_DVE_SPECS[op.name] = op.spec
    assert _dve_ops._SUB_OPCODE_FOR_NAME[op.name] < 0x20


HUBER_OP = _make_huber_op()
_register_op(HUBER_OP)


# --------------------------------------------------------------------------
# Bass program (SPMD, one program for all 8 cores)
# --------------------------------------------------------------------------
_dt = mybir.dt
_program_cache = {}


def _build_program(reps: int = 1):
    nc = bass.Bass()
    gtw = nc.declare_dram_parameter("gtw", [BL, 2 * FD], _dt.bfloat16, isOutput=False)
    prep = nc.declare_dram_parameter(
        "prep", [BL, 128, FD], _dt.bfloat16, isOutput=False
    )
    acc_out = nc.declare_dram_parameter(
        "acc", [128, BL * 8], _dt.float32, isOutput=True
    )

    with TileContext(nc) as tc:
        with (
            tc.tile_pool(name="w", bufs=3) as wpool,
            tc.tile_pool(name="p", bufs=3) as ppool,
            tc.tile_pool(name="s", bufs=2) as spool,
            tc.tile_pool(name="a", bufs=1) as apool,
        ):
            acc = apool.tile([128, BL * 8], _dt.float32)
            for _rep in range(reps):
                for b in range(BL):
                    w = wpool.tile([128, WW], _dt.bfloat16)
                    # staircase window: row x = gtflat2[b, 2x : 2x + WW]
                    nc.sync.dma_start(
                        out=w[:], in_=bass.AP(gtw, b * 2 * FD, [[2, 128], [1, WW]])
                    )
                    p = ppool.tile([128, FD], _dt.bfloat16)
                    nc.sync.dma_start(out=p[:], in_=prep[b])
                    for qi in range(8):
                        scr = spool.tile([128, FD], _dt.bfloat16)
                        col = b * 8 + qi
                        nc.vector._custom_dve(
                            HUBER_OP,
                            out=scr[:],
                            in0=w[:, 256 * qi : 256 * qi + FD],
                            in1=p[:],
                            accum_out=acc[:, col : col + 1],
                        )
            nc.sync.dma_start(out=acc_out[:], in_=acc[:])
    _split_multi_waits(nc)
    # Raw Bass (unlike Bacc.compile) never runs this pass; without it the
    # custom-DVE InstISA subclasses serialize with empty .instr bytes and
    # walrus fails with "ISA wrong length".
    mybir.codegen_inst_isa_subclasses(nc)
    return nc


def _get_program():
    if "nc" not in _program_cache:
        _program_cache["nc"] = _build_program()
    return _program_cache["nc"]


# --------------------------------------------------------------------------
# Host wrapper
# --------------------------------------------------------------------------
def _make_in_maps(pred: np.ndarray, gt: np.ndarray):
    pred = np.ascontiguousarray(pred, dtype=np.float32)
    gt = np.ascontiguousarray(gt, dtype=np.float32)
    in_maps = []
    for c in range(NCORES):
        sl = slice(c * BL, (c + 1) * BL)
        gtc = gt[sl]  # [BL, P, C]
        gtdup = np.concatenate([gtc, gtc], axis=1).reshape(BL, 2 * FD)
        gtdup = gtdup.astype(_bf16)
        predc = pred[sl].reshape(BL, 1, FD).astype(_bf16)
        prep = np.ascontiguousarray(np.broadcast_to(predc, (BL, 128, FD)))
        in_maps.append({"gtw": gtdup, "prep": prep})
    return in_maps


def _finish(results) -> np.float32:
    mins = []
    for c in range(NCORES):
        acc = np.asarray(results[c]["acc"], dtype=np.float32)  # [128, BL*8]
        acc = acc.reshape(128, BL, 8)  # [i_local, b, qi]
        dis = acc.transpose(1, 2, 0).reshape(BL, PNUM)  # [b, 128*qi + i_local]
        dis = dis / (2.0 * PNUM)
        mins.append(dis.min(axis=1))
    return np.asarray(np.mean(np.concatenate(mins)), dtype=np.float32)


def kernel(pred: np.ndarray, gt: np.ndarray) -> np.ndarray:
    nc = _get_program()
    in_maps = _make_in_maps(pred, gt)
    res = bass_utils.run_bass_kernel_spmd(nc, in_maps, list(range(NCORES)))
    return _finish(res.results)


# Exposed for test.py: run with tracing and return (value, BassKernelResults)
def kernel_traced(pred: np.ndarray, gt: np.ndarray, **kw):
    nc = _get_program()
    in_maps = _make_in_maps(pred, gt)
    res = bass_utils.run_bass_kernel_spmd(nc, in_maps, list(range(NCORES)), **kw)
    return _finish(res.results), res


# revision 10
# speedup vs baseline: 1.3077x; 1.3077x over previous
"""PolyMatchingLoss Trainium2 kernel.

Reference computation (B=128, P=1024, C=2):
    dis[b, i] = mean_j sum_c smooth_l1(pred[b,j,c] - gt[b,(i+j)%P,c])
    out = mean_b min_i dis[b, i]

Strategy:
  - Pure data parallel over batch: 16 batches per core x 8 cores.
  - Per (b, shift-block qi): one fused custom DVE instruction computes
    2*smooth_l1(W - P) elementwise over a [128 shifts, 2048 (j,c)] tile
    and reduces (sum) along the free axis into a [128,1] accumulator
    column.  smooth_l1 via m*(2t-m) = 2*huber, t=|d|, m=min(t,1).
  - The gt operand uses the "staircase" identity: W[x, y] = gtflat2[2x+y]
    (c-interleaved, cyclically duplicated gt).  One [128, 3840] window
    tile per (b) serves all 8 shift blocks as free-axis offset slices.
  - pred is broadcast along partitions (host-replicated).
  - min over shifts + mean over batch on host (tiny).
"""

from operator import add as _operator_add

import numpy as np
import ml_dtypes

_bf16 = ml_dtypes.bfloat16

from concourse import mybir
from concourse import bass, bass_utils
from concourse.tile import TileContext
import concourse.dve_ops as _dve_ops
from concourse.dve_ops import DveOp
from concourse.dve_spec import Spec, Src0, Src1, Zero, One, maxx, minn

# ---------------------------------------------------------------------------
# Workaround: this toolchain's walrus allows at most ONE sync wait per
# instruction; Tile emits 2+.  Split extras onto EventSemaphore carrier
# instructions inserted just before the offending instruction.
# ---------------------------------------------------------------------------
def _split_multi_waits(nc) -> int:
    n = 0
    for fn in nc.m.functions:
        for bb in fn.blocks:
            out = []
            for inst in bb.instructions:
                si = inst.sync_info
                if si is not None and si.on_wait and len(si.on_wait) > 1:
                    for k, w in enumerate(si.on_wait[:-1]):
                        out.append(
                            mybir.InstEventSemaphore(
                                name=f"{inst.name}_wsplit{k}",
                                opcode="EventSemaphore",
                                engine=inst.engine,
                                ins=[],
                                outs=[],
                                sync_info=mybir.SyncInfo(on_wait=[w], on_update=[]),
                            )
                        )
                        n += 1
                    si.on_wait = [si.on_wait[-1]]
                out.append(inst)
            bb.instructions = out
    return n


B = 128
PNUM = 1024
C = 2
NCORES = 8
BL = B // NCORES  # batches per core
FD = PNUM * C  # 2048 free elements per tile
WW = FD + 256 * 7  # 3840 window width


# --------------------------------------------------------------------------
# Custom DVE op: out = m*(2t - m) with t=|in0-in1|, m=min(t,1)  (= 2*huber)
#                accum_out = sum over free axis
# --------------------------------------------------------------------------
def _huber_ref(in0, in1, s0, s1, imm2):
    dd = in0.astype(np.float32) - in1.astype(np.float32)
    tt = np.abs(dd)
    mm = np.minimum(tt, 1.0)
    bb = (mm * (2.0 * tt - mm)).astype(np.float32)
    return bb, bb.reshape(bb.shape[0], -1).sum(axis=-1, keepdims=True)


def _make_huber_op() -> DveOp:
    d = Src0 - Src1
    nd = Src1 - Src0
    t = maxx(d, nd)
    m = minn(t, One)
    v = t - m
    w = t + v
    body = m * w
    return DveOp(
        "TENSOR_HUBER2_REDUCE",
        Spec(body=body, accum=_operator_add, accum_init=Zero, reference=_huber_ref),
        subdim=False,
        uops_sha={"v3": "e8f6160a1f1db788", "v4": "8b26f7daea78cb80"},
    )


def _register_op(op: DveOp) -> None:
    if op.name in _dve_ops._SUB_OPCODE_FOR_NAME:
        return
    _dve_ops.OPS.append(op)
    _dve_ops._SUB_OPCODE_FOR_NAME[op.name] = (
        _dve_ops._CUSTOM_DVE_ROW_BASE + len(_dve_ops.OPS) - 1
    )
    _dve_ops.CUSTOM_DVE_SPECS[op.name] = op.spec
    assert _dve_ops._SUB_OPCODE_FOR_NAME[op.name] < 0x20


HUBER_OP = _make_huber_op()
_register_op(HUBER_OP)


# --------------------------------------------------------------------------
# Bass program (SPMD, one program for all 8 cores)
# --------------------------------------------------------------------------
_dt = mybir.dt
_program_cache = {}


def _build_program(reps: int = 1):
    nc = bass.Bass()
    gtw = nc.declare_dram_parameter("gtw", [BL, 2 * FD], _dt.bfloat16, isOutput=False)
    prep = nc.declare_dram_parameter(
        "prep", [BL, 128, FD], _dt.bfloat16, isOutput=False
    )
    acc_out = nc.declare_dram_parameter(
        "acc", [128, BL * 8], _dt.float32, isOutput=True
    )

    with TileContext(nc) as tc:
        with (
            tc.tile_pool(name="w", bufs=3) as wpool,
            tc.tile_pool(name="p", bufs=3) as ppool,
            tc.tile_pool(name="s", bufs=2) as spool,
            tc.tile_pool(name="a", bufs=1) as apool,
        ):
            acc = apool.tile([128, BL * 8], _dt.float32)
            for _rep in range(reps):
                for b in range(BL):
                    w = wpool.tile([128, WW], _dt.bfloat16)
                    # staircase window: row x = gtflat2[b, 2x : 2x + WW]
                    nc.sync.dma_start(
                        out=w[:], in_=bass.AP(gtw, b * 2 * FD, [[2, 128], [1, WW]])
                    )
                    p = ppool.tile([128, FD], _dt.bfloat16)
                    nc.sync.dma_start(out=p[:], in_=prep[b])
                    for qi in range(8):
                        scr = spool.tile([128, FD], _dt.float32)
                        col = b * 8 + qi
                        nc.vector._custom_dve(
                            HUBER_OP,
                            out=scr[:],
                            in0=w[:, 256 * qi : 256 * qi + FD],
                            in1=p[:],
                            accum_out=acc[:, col : col + 1],
                        )
            nc.sync.dma_start(out=acc_out[:], in_=acc[:])
    _split_multi_waits(nc)
    # Raw Bass (unlike Bacc.compile) never runs this pass; without it the
    # custom-DVE InstISA subclasses serialize with empty .instr bytes and
    # walrus fails with "ISA wrong length".
    mybir.codegen_inst_isa_subclasses(nc)
    return nc


def _get_program():
    if "nc" not in _program_cache:
        _program_cache["nc"] = _build_program()
    return _program_cache["nc"]


# --------------------------------------------------------------------------
# Host wrapper
# --------------------------------------------------------------------------
def _make_in_maps(pred: np.ndarray, gt: np.ndarray):
    pred = np.ascontiguousarray(pred, dtype=np.float32)
    gt = np.ascontiguousarray(gt, dtype=np.float32)
    in_maps = []
    for c in range(NCORES):
        sl = slice(c * BL, (c + 1) * BL)
        gtc = gt[sl]  # [BL, P, C]
        gtdup = np.concatenate([gtc, gtc], axis=1).reshape(BL, 2 * FD)
        gtdup = gtdup.astype(_bf16)
        predc = pred[sl].reshape(BL, 1, FD).astype(_bf16)
        prep = np.ascontiguousarray(np.broadcast_to(predc, (BL, 128, FD)))
        in_maps.append({"gtw": gtdup, "prep": prep})
    return in_maps


def _finish(results) -> np.float32:
    mins = []
    for c in range(NCORES):
        acc = np.asarray(results[c]["acc"], dtype=np.float32)  # [128, BL*8]
        acc = acc.reshape(128, BL, 8)  # [i_local, b, qi]
        dis = acc.transpose(1, 2, 0).reshape(BL, PNUM)  # [b, 128*qi + i_local]
        dis = dis / (2.0 * PNUM)
        mins.append(dis.min(axis=1))
    return np.asarray(np.mean(np.concatenate(mins)), dtype=np.float32)


def kernel(pred: np.ndarray, gt: np.ndarray) -> np.ndarray:
    nc = _get_program()
    in_maps = _make_in_maps(pred, gt)
    res = bass_utils.run_bass_kernel_spmd(nc, in_maps, list(range(NCORES)))
    return _finish(res.results)


# Exposed for test.py: run with tracing and return (value, BassKernelResults)
def kernel_traced(pred: np.ndarray, gt: np.ndarray, **kw):
    nc = _get_program()
    in_maps = _make_in_maps(pred, gt)
    res = bass_utils.run_bass_kernel_spmd(nc, in_maps, list(range(NCORES)), **kw)
    return _finish(res.results), res


# revision 11
# speedup vs baseline: 1.8034x; 1.3791x over previous
"""PolyMatchingLoss Trainium2 kernel.

Reference computation (B=128, P=1024, C=2):
    dis[b, i] = mean_j sum_c smooth_l1(pred[b,j,c] - gt[b,(i+j)%P,c])
    out = mean_b min_i dis[b, i]

Strategy:
  - Pure data parallel over batch: 16 batches per core x 8 cores.
  - Per (b, shift-block qi): one fused custom DVE instruction computes
    2*smooth_l1(W - P) elementwise over a [128 shifts, 2048 (j,c)] tile
    and reduces (sum) along the free axis into a [128,1] accumulator
    column.  smooth_l1 via m*(2t-m) = 2*huber, t=|d|, m=min(t,1).
  - The gt operand uses the "staircase" identity: W[x, y] = gtflat2[2x+y]
    (c-interleaved, cyclically duplicated gt).  One [128, 3840] window
    tile per (b) serves all 8 shift blocks as free-axis offset slices.
  - pred is broadcast along partitions (host-replicated).
  - min over shifts + mean over batch on host (tiny).
"""

from operator import add as _operator_add

import numpy as np
import ml_dtypes

_bf16 = ml_dtypes.bfloat16

from concourse import mybir
from concourse import bass, bass_utils
from concourse.tile import TileContext
import concourse.dve_ops as _dve_ops
from concourse.dve_ops import DveOp
from concourse.dve_spec import Spec, Src0, Src1, Zero, One, maxx, minn

# ---------------------------------------------------------------------------
# Workaround: this toolchain's walrus allows at most ONE sync wait per
# instruction; Tile emits 2+.  Split extras onto EventSemaphore carrier
# instructions inserted just before the offending instruction.
# ---------------------------------------------------------------------------
def _split_multi_waits(nc) -> int:
    n = 0
    for fn in nc.m.functions:
        for bb in fn.blocks:
            out = []
            for inst in bb.instructions:
                si = inst.sync_info
                if si is not None and si.on_wait and len(si.on_wait) > 1:
                    for k, w in enumerate(si.on_wait[:-1]):
                        out.append(
                            mybir.InstEventSemaphore(
                                name=f"{inst.name}_wsplit{k}",
                                opcode="EventSemaphore",
                                engine=inst.engine,
                                ins=[],
                                outs=[],
                                sync_info=mybir.SyncInfo(on_wait=[w], on_update=[]),
                            )
                        )
                        n += 1
                    si.on_wait = [si.on_wait[-1]]
                out.append(inst)
            bb.instructions = out
    return n


B = 128
PNUM = 1024
C = 2
NCORES = 8
BL = B // NCORES  # batches per core
FD = PNUM * C  # 2048 free elements per tile
WW = FD + 256 * 7  # 3840 window width


# --------------------------------------------------------------------------
# Custom DVE op: out = m*(2t - m) with t=|in0-in1|, m=min(t,1)  (= 2*huber)
#                accum_out = sum over free axis
# --------------------------------------------------------------------------
def _huber_ref(in0, in1, s0, s1, imm2):
    dd = in0.astype(np.float32) - in1.astype(np.float32)
    tt = np.abs(dd)
    mm = np.minimum(tt, 1.0)
    bb = (mm * (2.0 * tt - mm)).astype(np.float32)
    return bb, bb.reshape(bb.shape[0], -1).sum(axis=-1, keepdims=True)


def _make_huber_op() -> DveOp:
    d = Src0 - Src1
    nd = Src1 - Src0
    t = maxx(d, nd)
    m = minn(t, One)
    v = t - m
    w = t + v
    body = m * w
    return DveOp(
        "TENSOR_HUBER2_REDUCE",
        Spec(body=body, accum=_operator_add, accum_init=Zero, reference=_huber_ref),
        subdim=False,
        uops_sha={"v3": "e8f6160a1f1db788", "v4": "8b26f7daea78cb80"},
    )


def _register_op(op: DveOp) -> None:
    if op.name in _dve_ops._SUB_OPCODE_FOR_NAME:
        return
    _dve_ops.OPS.append(op)
    _dve_ops._SUB_OPCODE_FOR_NAME[op.name] = (
        _dve_ops._CUSTOM_DVE_ROW_BASE + len(_dve_ops.OPS) - 1
    )
    _dve_ops.CUSTOM_DVE_SPECS[op.name] = op.spec
    assert _dve_ops._SUB_OPCODE_FOR_NAME[op.name] < 0x20


HUBER_OP = _make_huber_op()
_register_op(HUBER_OP)


# --------------------------------------------------------------------------
# Bass program (SPMD, one program for all 8 cores)
# --------------------------------------------------------------------------
_dt = mybir.dt
_program_cache = {}


def _build_program(reps: int = 1):
    nc = bass.Bass()
    gtw = nc.declare_dram_parameter("gtw", [BL, 2 * FD], _dt.float32, isOutput=False)
    prep = nc.declare_dram_parameter(
        "prep", [BL, 128, FD], _dt.float32, isOutput=False
    )
    acc_out = nc.declare_dram_parameter(
        "acc", [128, BL * 8], _dt.float32, isOutput=True
    )

    with TileContext(nc) as tc:
        with (
            tc.tile_pool(name="w", bufs=3) as wpool,
            tc.tile_pool(name="p", bufs=3) as ppool,
            tc.tile_pool(name="s", bufs=2) as spool,
            tc.tile_pool(name="a", bufs=1) as apool,
        ):
            acc = apool.tile([128, BL * 8], _dt.float32)
            for _rep in range(reps):
                for b in range(BL):
                    w = wpool.tile([128, WW], _dt.float32)
                    # staircase window: row x = gtflat2[b, 2x : 2x + WW]
                    nc.sync.dma_start(
                        out=w[:], in_=bass.AP(gtw, b * 2 * FD, [[2, 128], [1, WW]])
                    )
                    p = ppool.tile([128, FD], _dt.float32)
                    nc.sync.dma_start(out=p[:], in_=prep[b])
                    for qi in range(8):
                        scr = spool.tile([128, FD], _dt.float32)
                        col = b * 8 + qi
                        nc.vector._custom_dve(
                            HUBER_OP,
                            out=scr[:],
                            in0=w[:, 256 * qi : 256 * qi + FD],
                            in1=p[:],
                            accum_out=acc[:, col : col + 1],
                        )
            nc.sync.dma_start(out=acc_out[:], in_=acc[:])
    _split_multi_waits(nc)
    # Raw Bass (unlike Bacc.compile) never runs this pass; without it the
    # custom-DVE InstISA subclasses serialize with empty .instr bytes and
    # walrus fails with "ISA wrong length".
    mybir.codegen_inst_isa_subclasses(nc)
    return nc


def _get_program():
    if "nc" not in _program_cache:
        _program_cache["nc"] = _build_program()
    return _program_cache["nc"]


# --------------------------------------------------------------------------
# Host wrapper
# --------------------------------------------------------------------------
def _make_in_maps(pred: np.ndarray, gt: np.ndarray):
    pred = np.ascontiguousarray(pred, dtype=np.float32)
    gt = np.ascontiguousarray(gt, dtype=np.float32)
    in_maps = []
    for c in range(NCORES):
        sl = slice(c * BL, (c + 1) * BL)
        gtc = gt[sl]  # [BL, P, C]
        gtdup = np.concatenate([gtc, gtc], axis=1).reshape(BL, 2 * FD)
        predc = pred[sl].reshape(BL, 1, FD)
        prep = np.ascontiguousarray(np.broadcast_to(predc, (BL, 128, FD)))
        in_maps.append({"gtw": gtdup, "prep": prep})
    return in_maps


def _finish(results) -> np.float32:
    mins = []
    for c in range(NCORES):
        acc = np.asarray(results[c]["acc"], dtype=np.float32)  # [128, BL*8]
        acc = acc.reshape(128, BL, 8)  # [i_local, b, qi]
        dis = acc.transpose(1, 2, 0).reshape(BL, PNUM)  # [b, 128*qi + i_local]
        dis = dis / (2.0 * PNUM)
        mins.append(dis.min(axis=1))
    return np.asarray(np.mean(np.concatenate(mins)), dtype=np.float32)


def kernel(pred: np.ndarray, gt: np.ndarray) -> np.ndarray:
    nc = _get_program()
    in_maps = _make_in_maps(pred, gt)
    res = bass_utils.run_bass_kernel_spmd(nc, in_maps, list(range(NCORES)))
    return _finish(res.results)


# Exposed for test.py: run with tracing and return (value, BassKernelResults)
def kernel_traced(pred: np.ndarray, gt: np.ndarray, **kw):
    nc = _get_program()
    in_maps = _make_in_maps(pred, gt)
    res = bass_utils.run_bass_kernel_spmd(nc, in_maps, list(range(NCORES)), **kw)
    return _finish(res.results), res
